# revision 8
# baseline (speedup 1.0000x reference)
"""AttnDecoderLSTM Trainium2 kernel: 8-core data-parallel (batch sharded).

Raw-Bass implementation (explicit semaphores; the container's walrus build
rejects TileContext's multi-wait instructions).  Each core handles 64
batches.  Host pre-transposes weights and ctx so every matmul contraction
sits on the partition axis.

Pipeline per core:
  LSTM gates (weights streamed, double-buffered) -> c1/h1 elementwise ->
  h1 transpose -> target = h1 @ W_in.T (transposed layout) ->
  scores via ctxT stream (per-batch matmul, target column stationary) ->
  masked softmax -> alpha transpose -> weighted sum via ctx stream
  (ctx tile stationary, alpha column moving; columns land in psum) ->
  h_tilde = tanh(cat @ W_out.T) -> logitT = W_dec @ h_tilde.T + b_dec.
"""

import sys

sys.path.insert(0, "/opt/trn_rl_repo")

from contextlib import ExitStack

import numpy as np

import concourse.bass as bass
from concourse import mybir

B, S = 512, 256
HID = 1024
EMB, FEAT = 256, 512
XDIM = EMB + FEAT  # 768
G4 = 4 * HID
OUT_ACT = 14
NCORES = 8
BL = B // NCORES  # 64

KX = XDIM // 128  # 6
KH = HID // 128  # 8

F32 = mybir.dt.float32
AF = mybir.ActivationFunctionType
ALU = mybir.AluOpType
AX = mybir.AxisListType

_STATE = {}


def build_nc():
    nc = bass.Bass("TRN2")

    xT = nc.declare_dram_parameter("xT", [XDIM, BL], F32, isOutput=False)
    h0T = nc.declare_dram_parameter("h0T", [HID, BL], F32, isOutput=False)
    c0 = nc.declare_dram_parameter("c0", [BL, HID], F32, isOutput=False)
    maskadd = nc.declare_dram_parameter("maskadd", [BL, S], F32, isOutput=False)
    ctxT_d = nc.declare_dram_parameter("ctxT", [BL, HID, S], F32, isOutput=False)
    ctx_d = nc.declare_dram_parameter("ctx", [BL, S, HID], F32, isOutput=False)
    W_ihT = nc.declare_dram_parameter("W_ihT", [XDIM, G4], F32, isOutput=False)
    W_hhTa = nc.declare_dram_parameter("W_hhTa", [HID + 1, G4], F32, isOutput=False)
    W_inT = nc.declare_dram_parameter("W_inT", [HID, HID], F32, isOutput=False)
    W_outT = nc.declare_dram_parameter("W_outT", [2 * HID, HID], F32, isOutput=False)
    W_decT = nc.declare_dram_parameter("W_decT", [HID, OUT_ACT], F32, isOutput=False)
    bdec = nc.declare_dram_parameter("bdec", [OUT_ACT, 1], F32, isOutput=False)
    identD = nc.declare_dram_parameter("identD", [128, 128], F32, isOutput=False)
    onesD = nc.declare_dram_parameter("onesD", [1, BL], F32, isOutput=False)

    h1_out = nc.declare_dram_parameter("h1_out", [BL, HID], F32, isOutput=True)
    c1_out = nc.declare_dram_parameter("c1_out", [BL, HID], F32, isOutput=True)
    alpha_out = nc.declare_dram_parameter("alpha_out", [BL, S], F32, isOutput=True)
    logitT_out = nc.declare_dram_parameter(
        "logitT_out", [OUT_ACT, BL], F32, isOutput=True
    )

    es = ExitStack()
    with es:
        counter = [0]

        def sb(shape):
            counter[0] += 1
            return es.enter_context(nc.sbuf_tensor(f"t{counter[0]}", shape, F32))

        def pst(shape):
            counter[0] += 1
            return es.enter_context(nc.psum_tensor(f"p{counter[0]}", shape, F32))

        def sem(name):
            return es.enter_context(nc.semaphore(name))

        # resident SBUF
        xT_sb = sb([128, KX, BL])
        h0T_sb = sb([128, KH, BL])
        c0_sb = sb([BL, HID])
        mask_sb = sb([BL, S])
        bdec_sb = sb([OUT_ACT, 1])
        ones_sb = sb([1, BL])
        ident_sb = sb([128, 128])
        gact = sb([BL, 8, 512])
        t1 = sb([BL, 512])
        t2 = sb([BL, 512])
        t3 = sb([BL, 512])
        c1_sb = sb([BL, HID])
        h1_sb = sb([BL, HID])
        h1T_sb = sb([128, KH, BL])
        tgtT_sb = sb([128, KH, BL])
        scores_sb = sb([BL, S])
        alpha_sb = sb([BL, S])
        alphaT_sb = sb([128, 2, BL])
        catT_sb = sb([128, KH, BL])
        ht_sb = sb([BL, HID])
        htT_sb = sb([128, KH, BL])
        nm_sb = sb([BL, 1])
        negm_sb = sb([BL, 1])
        ssum_sb = sb([BL, 1])
        rinv_sb = sb([BL, 1])
        logitT_sb = sb([OUT_ACT, BL])
        # streamed buffers (double)
        wbuf = [sb([128, 16, 512]) for _ in range(2)]
        waug = [sb([1, 512]) for _ in range(2)]
        winb = [sb([128, KH, 128]) for _ in range(2)]
        ctb = [sb([128, KH, S]) for _ in range(2)]
        cxb = [sb([128, 2, HID]) for _ in range(2)]
        stage = [sb([1, S]) for _ in range(2)]
        wdec_sb = sb([128, KH, OUT_ACT])

        # PSUM (8 banks total)
        psG = [pst([BL, 512]) for _ in range(2)]
        psT = [pst([128, BL]) for _ in range(2)]
        psS = [pst([1, S]) for _ in range(2)]
        psW = [pst([128, KH]) for _ in range(2)]

        # sems
        dIn = sem("dIn")
        dW = [sem("dW0"), sem("dW1")]
        dWi = [sem("dWi0"), sem("dWi1")]
        dCt = [sem("dCt0"), sem("dCt1")]
        dSt = [sem("dSt0"), sem("dSt1")]
        dCx = [sem("dCx0"), sem("dCx1")]
        dWo = [sem("dWo0"), sem("dWo1")]
        dDec = sem("dDec")
        dOut = sem("dOut")
        peG = sem("peG")
        aG = sem("aG")
        vC = sem("vC")
        aT = sem("aT")
        vH = sem("vH")
        pePT = sem("pePT")
        aPT = sem("aPT")
        peS = sem("peS")
        aSc = sem("aSc")
        vSm1 = sem("vSm1")
        aExp = sem("aExp")
        vAl = sem("vAl")
        peW = sem("peW")
        vW = sem("vW")
        vLg = sem("vLg")

        block = es.enter_context(nc.Block())

        @block.sync
        def _(sync):
            # one-time inputs: 7 DMAs -> dIn (total 112)
            sync.dma_start(
                out=xT_sb[:], in_=xT.rearrange("(kc p) b -> p kc b", p=128)
            ).then_inc(dIn, 16)
            sync.dma_start(
                out=h0T_sb[:], in_=h0T.rearrange("(kc p) b -> p kc b", p=128)
            ).then_inc(dIn, 16)
            sync.dma_start(out=c0_sb[:], in_=c0[:]).then_inc(dIn, 16)
            sync.dma_start(out=mask_sb[:], in_=maskadd[:]).then_inc(dIn, 16)
            sync.dma_start(out=bdec_sb[:], in_=bdec[:]).then_inc(dIn, 16)
            sync.dma_start(out=ident_sb[:], in_=identD[:]).then_inc(dIn, 16)
            sync.dma_start(out=ones_sb[:], in_=onesD[:]).then_inc(dIn, 16)
            # LSTM weight stream: chunk n -> wbuf[n%2] (15 DMAs each)
            for n in range(8):
                p = n % 2
                if n >= 2:
                    sync.wait_ge(peG, n - 1)
                for kc in range(KX):
                    sync.dma_start(
                        out=wbuf[p][:, kc, :],
                        in_=W_ihT[kc * 128:(kc + 1) * 128, n * 512:(n + 1) * 512],
                    ).then_inc(dW[p], 16)
                for kc in range(KH):
                    sync.dma_start(
                        out=wbuf[p][:, KX + kc, :],
                        in_=W_hhTa[kc * 128:(kc + 1) * 128, n * 512:(n + 1) * 512],
                    ).then_inc(dW[p], 16)
                sync.dma_start(
                    out=waug[p][:], in_=W_hhTa[HID:HID + 1, n * 512:(n + 1) * 512]
                ).then_inc(dW[p], 16)
            # W_inT stream: m-chunk -> winb[m%2] (8 DMAs each)
            for m in range(8):
                p = m % 2
                if m >= 2:
                    sync.wait_ge(pePT, m + 7)  # targetT j=8+(m-2) consumed
                for kc in range(KH):
                    sync.dma_start(
                        out=winb[p][:, kc, :],
                        in_=W_inT[kc * 128:(kc + 1) * 128, m * 128:(m + 1) * 128],
                    ).then_inc(dWi[p], 16)
            # scores: ctxT stream + stage row evacuation
            for b in range(2):
                sync.dma_start(
                    out=ctb[b][:],
                    in_=ctxT_d[b].rearrange("(kc p) s -> p kc s", p=128),
                ).then_inc(dCt[b], 16)
            for b in range(BL):
                if b + 2 < BL:
                    sync.wait_ge(peS, b + 1)
                    sync.dma_start(
                        out=ctb[b % 2][:],
                        in_=ctxT_d[b + 2].rearrange("(kc p) s -> p kc s", p=128),
                    ).then_inc(dCt[b % 2], 16)
                sync.wait_ge(aSc, b + 1)
                sync.dma_start(
                    out=scores_sb[b:b + 1, :], in_=stage[b % 2][:]
                ).then_inc(dSt[b % 2], 16)
            # weighted: ctx stream
            for b in range(2):
                sync.dma_start(
                    out=cxb[b][:],
                    in_=ctx_d[b].rearrange("(sc p) d -> p sc d", p=128),
                ).then_inc(dCx[b], 16)
            for b in range(BL):
                if b + 2 < BL:
                    sync.wait_ge(peW, b + 1)
                    sync.dma_start(
                        out=cxb[b % 2][:],
                        in_=ctx_d[b + 2].rearrange("(sc p) d -> p sc d", p=128),
                    ).then_inc(dCx[b % 2], 16)
            # W_out stream: chunk n -> wbuf[n] (16 DMAs each)
            for n in range(2):
                sync.wait_ge(peG, 7 + n)
                for kc in range(16):
                    sync.dma_start(
                        out=wbuf[n][:, kc, :],
                        in_=W_outT[kc * 128:(kc + 1) * 128, n * 512:(n + 1) * 512],
                    ).then_inc(dWo[n], 16)
            # W_dec (one-shot)
            for kc in range(KH):
                sync.dma_start(
                    out=wdec_sb[:, kc, :], in_=W_decT[kc * 128:(kc + 1) * 128, :]
                ).then_inc(dDec, 16)
            # outputs
            sync.wait_ge(vC, 2)
            sync.dma_start(out=c1_out[:], in_=c1_sb[:]).then_inc(dOut, 16)
            sync.wait_ge(vH, 2)
            sync.dma_start(out=h1_out[:], in_=h1_sb[:]).then_inc(dOut, 16)
            sync.wait_ge(vAl, 1)
            sync.dma_start(out=alpha_out[:], in_=alpha_sb[:]).then_inc(dOut, 16)
            sync.wait_ge(vLg, 1)
            sync.dma_start(out=logitT_out[:], in_=logitT_sb[:]).then_inc(dOut, 16)

        @block.tensor
        def _(pe):
            pe.wait_ge(dIn, 112)
            # LSTM gates: 8 n-chunks x (6 + 8 + 1) matmuls
            for n in range(8):
                p = n % 2
                pe.wait_ge(dW[p], 240 * (n // 2 + 1))
                if n >= 2:
                    pe.wait_ge(aG, n - 1)
                for kc in range(KX):
                    pe.matmul(
                        psG[p][:], xT_sb[:, kc, :], wbuf[p][:, kc, :],
                        start=(kc == 0), stop=False,
                    )
                for kc in range(KH):
                    pe.matmul(
                        psG[p][:], h0T_sb[:, kc, :], wbuf[p][:, KX + kc, :],
                        start=False, stop=False,
                    )
                pe.matmul(
                    psG[p][:], ones_sb[:], waug[p][:], start=False, stop=True
                ).then_inc(peG, 1)
            # h1 transposes: psT uses j = 0..7
            pe.wait_ge(vH, 2)
            for j in range(KH):
                if j >= 2:
                    pe.wait_ge(aPT, j - 1)
                pe.transpose(
                    psT[j % 2][:], h1_sb[:, j * 128:(j + 1) * 128],
                    ident_sb[:BL, :BL],
                ).then_inc(pePT, 1)
            # targetT: psT uses j = 8..15
            for m in range(8):
                j = 8 + m
                pe.wait_ge(dWi[m % 2], 128 * (m // 2 + 1))
                pe.wait_ge(aPT, max(j - 1, 8))
                for kc in range(KH):
                    mm = pe.matmul(
                        psT[j % 2][:], winb[m % 2][:, kc, :], h1T_sb[:, kc, :],
                        start=(kc == 0), stop=(kc == KH - 1),
                    )
                mm.then_inc(pePT, 1)
            # scores
            pe.wait_ge(aPT, 16)
            for b in range(BL):
                pe.wait_ge(dCt[b % 2], 16 * (b // 2 + 1))
                if b >= 2:
                    pe.wait_ge(aSc, b - 1)
                for kc in range(KH):
                    mm = pe.matmul(
                        psS[b % 2][:], tgtT_sb[:, kc, b:b + 1], ctb[b % 2][:, kc, :],
                        start=(kc == 0), stop=(kc == KH - 1),
                    )
                mm.then_inc(peS, 1)
            # alphaT transposes: psT uses j = 16, 17
            pe.wait_ge(vAl, 1)
            for sc in range(2):
                j = 16 + sc
                pe.wait_ge(aPT, j - 1)
                pe.transpose(
                    psT[j % 2][:], alpha_sb[:, sc * 128:(sc + 1) * 128],
                    ident_sb[:BL, :BL],
                ).then_inc(pePT, 1)
            # weighted: ctx tile stationary, alpha column moving
            pe.wait_ge(aPT, 18)
            for b in range(BL):
                pe.wait_ge(dCx[b % 2], 16 * (b // 2 + 1))
                if b >= 2:
                    pe.wait_ge(vW, b - 1)
                for dc in range(KH):
                    for sc in range(2):
                        mm = pe.matmul(
                            psW[b % 2][:, dc:dc + 1],
                            cxb[b % 2][:, sc, dc * 128:(dc + 1) * 128],
                            alphaT_sb[:, sc, b:b + 1],
                            start=(sc == 0), stop=(sc == 1),
                        )
                mm.then_inc(peW, 1)
            # W_out: h_tilde chunks into psG (peG/aG continue: 9, 10)
            pe.wait_ge(vW, BL)
            for n in range(2):
                pe.wait_ge(dWo[n], 256)
                for kc in range(16):
                    lhsT = catT_sb[:, kc, :] if kc < KH else h1T_sb[:, kc - KH, :]
                    mm = pe.matmul(
                        psG[n][:], lhsT, wbuf[n][:, kc, :],
                        start=(kc == 0), stop=(kc == 15),
                    )
                mm.then_inc(peG, 1)
            # ht transposes: psT uses j = 18..25
            pe.wait_ge(aG, 10)
            for dc in range(KH):
                j = 18 + dc
                pe.wait_ge(aPT, j - 1)
                pe.transpose(
                    psT[j % 2][:], ht_sb[:, dc * 128:(dc + 1) * 128],
                    ident_sb[:BL, :BL],
                ).then_inc(pePT, 1)
            # logitT: psT use j = 26
            pe.wait_ge(dDec, 128)
            pe.wait_ge(aPT, 26)
            for kc in range(KH):
                mm = pe.matmul(
                    psT[0][:OUT_ACT, :], wdec_sb[:, kc, :], htT_sb[:, kc, :],
                    start=(kc == 0), stop=(kc == KH - 1),
                )
            mm.then_inc(pePT, 1)

        @block.scalar
        def _(sc_e):
            # gate nonlinearities (evac psG)
            for n in range(8):
                sc_e.wait_ge(peG, n + 1)
                func = AF.Tanh if n // 2 == 2 else AF.Sigmoid
                sc_e.activation(gact[:, n, :], psG[n % 2][:], func).then_inc(aG, 1)
            # tanh(c1) halves
            for hc in range(2):
                sc_e.wait_ge(vC, hc + 1)
                if hc == 1:
                    sc_e.wait_ge(vH, 1)
                sc_e.activation(
                    t3[:], c1_sb[:, hc * 512:(hc + 1) * 512], AF.Tanh
                ).then_inc(aT, 1)
            # psT evacuations j = 0..25 with interleaved phases
            for j in range(26):
                if j < 8:
                    dest = h1T_sb[:, j, :]
                elif j < 16:
                    dest = tgtT_sb[:, j - 8, :]
                elif j < 18:
                    dest = alphaT_sb[:, j - 16, :]
                else:
                    dest = htT_sb[:, j - 18, :]
                sc_e.wait_ge(pePT, j + 1)
                sc_e.activation(dest, psT[j % 2][:], AF.Copy).then_inc(aPT, 1)
                if j == 15:
                    # scores stage copies
                    for b in range(BL):
                        sc_e.wait_ge(peS, b + 1)
                        if b >= 2:
                            sc_e.wait_ge(dSt[b % 2], 16 * (b // 2))
                        sc_e.activation(
                            stage[b % 2][:], psS[b % 2][:], AF.Copy
                        ).then_inc(aSc, 1)
                    # exp (after vector masked-max)
                    sc_e.wait_ge(vSm1, 1)
                    sc_e.activation(
                        alpha_sb[:], scores_sb[:], AF.Exp,
                        bias=negm_sb[:], scale=1.0, accum_out=ssum_sb[:],
                    ).then_inc(aExp, 1)
                if j == 17:
                    # W_out tanh evacs (peG 9, 10 -> aG 9, 10)
                    for n in range(2):
                        sc_e.wait_ge(peG, 9 + n)
                        sc_e.activation(
                            ht_sb[:, n * 512:(n + 1) * 512], psG[n][:], AF.Tanh
                        ).then_inc(aG, 1)

        @block.vector
        def _(ve):
            ve.wait_ge(dIn, 112)
            ve.wait_ge(aG, 8)
            # c1 = f*c0 + i*g ; h1 = o*tanh(c1)
            for hc in range(2):
                sl = slice(hc * 512, (hc + 1) * 512)
                if hc == 1:
                    ve.drain()
                ve.tensor_mul(t1[:], gact[:, 0 + hc, :], gact[:, 4 + hc, :])
                ve.tensor_mul(t2[:], gact[:, 2 + hc, :], c0_sb[:, sl])
                ve.drain()
                ve.tensor_add(c1_sb[:, sl], t1[:], t2[:]).then_inc(vC, 1)
                ve.wait_ge(aT, hc + 1)
                ve.tensor_mul(h1_sb[:, sl], gact[:, 6 + hc, :], t3[:]).then_inc(
                    vH, 1
                )
            # masked softmax prologue
            ve.wait_ge(dSt[0], 16 * (BL // 2))
            ve.wait_ge(dSt[1], 16 * (BL // 2))
            ve.tensor_add(scores_sb[:], scores_sb[:], mask_sb[:])
            ve.drain()
            ve.tensor_reduce(nm_sb[:], scores_sb[:], axis=AX.X, op=ALU.max)
            ve.drain()
            ve.tensor_scalar_mul(negm_sb[:], nm_sb[:], -1.0).then_inc(vSm1, 1)
            # normalize
            ve.wait_ge(aExp, 1)
            ve.reciprocal(rinv_sb[:], ssum_sb[:])
            ve.drain()
            ve.tensor_scalar_mul(alpha_sb[:], alpha_sb[:], rinv_sb[:]).then_inc(
                vAl, 1
            )
            # weighted evacuations: psW columns -> catT
            for b in range(BL):
                ve.wait_ge(peW, b + 1)
                ve.tensor_copy(catT_sb[:, 0:KH, b], psW[b % 2][:]).then_inc(vW, 1)
            # logit bias add
            ve.wait_ge(pePT, 27)
            ve.tensor_scalar_add(
                logitT_sb[:], psT[0][:OUT_ACT, :], bdec_sb[:]
            ).then_inc(vLg, 1)

    return nc


def _prep_inputs(inputs):
    action = np.asarray(inputs["action"]).reshape(-1).astype(np.int64)
    feature = np.asarray(inputs["feature"], dtype=np.float32)
    h_0 = np.asarray(inputs["h_0"], dtype=np.float32)
    c_0 = np.asarray(inputs["c_0"], dtype=np.float32)
    ctx = np.asarray(inputs["ctx"], dtype=np.float32)
    ctx_mask = np.asarray(inputs["ctx_mask"])
    emb_tab = np.asarray(inputs["embedding"], dtype=np.float32)
    W_ih = np.asarray(inputs["W_ih"], dtype=np.float32)
    W_hh = np.asarray(inputs["W_hh"], dtype=np.float32)
    b_ih = np.asarray(inputs["b_ih"], dtype=np.float32)
    b_hh = np.asarray(inputs["b_hh"], dtype=np.float32)
    W_in = np.asarray(inputs["W_in"], dtype=np.float32)
    W_out = np.asarray(inputs["W_out"], dtype=np.float32)
    W_dec = np.asarray(inputs["W_dec"], dtype=np.float32)
    b_dec = np.asarray(inputs["b_dec"], dtype=np.float32)

    x = np.concatenate([emb_tab[action], feature], axis=1)  # [B, 768]
    xT = np.ascontiguousarray(x.T)
    h0T = np.ascontiguousarray(h_0.T)
    maskadd = np.where(ctx_mask, np.float32(-1e30), np.float32(0.0)).astype(
        np.float32
    )

    shared = {
        "W_ihT": np.ascontiguousarray(W_ih.T),
        "W_hhTa": np.ascontiguousarray(np.vstack([W_hh.T, (b_ih + b_hh)[None, :]])),
        "W_inT": np.ascontiguousarray(W_in.T),
        "W_outT": np.ascontiguousarray(W_out.T),
        "W_decT": np.ascontiguousarray(W_dec.T),
        "bdec": np.ascontiguousarray(b_dec.reshape(OUT_ACT, 1)),
        "identD": np.eye(128, dtype=np.float32),
        "onesD": np.ones((1, BL), np.float32),
    }
    in_maps = []
    for i in range(NCORES):
        sl = slice(i * BL, (i + 1) * BL)
        ctx_i = np.ascontiguousarray(ctx[sl])
        m = {
            "xT": np.ascontiguousarray(xT[:, sl]),
            "h0T": np.ascontiguousarray(h0T[:, sl]),
            "c0": np.ascontiguousarray(c_0[sl]),
            "maskadd": np.ascontiguousarray(maskadd[sl]),
            "ctx": ctx_i,
            "ctxT": np.ascontiguousarray(ctx_i.transpose(0, 2, 1)),
        }
        m.update(shared)
        in_maps.append(m)
    return in_maps


def kernel(**inputs):
    from concourse.bass_utils import run_bass_kernel_spmd

    if "nc" not in _STATE:
        _STATE["nc"] = build_nc()
    nc = _STATE["nc"]
    in_maps = _prep_inputs(inputs)
    res = run_bass_kernel_spmd(nc, in_maps, core_ids=list(range(NCORES)))
    outs = res.results
    _STATE["last_exec_time_ns"] = res.exec_time_ns
    h_1 = np.concatenate([o["h1_out"] for o in outs], axis=0)
    c_1 = np.concatenate([o["c1_out"] for o in outs], axis=0)
    alpha = np.concatenate([o["alpha_out"] for o in outs], axis=0)
    logit = np.concatenate(
        [np.ascontiguousarray(o["logitT_out"].T) for o in outs], axis=0
    )
    return (h_1, c_1, alpha, logit)
